# revision 15
# baseline (speedup 1.0000x reference)
"""ClassAlignmentLoss kernel for 8 TRN2 NeuronCores (Bass/Tile).

Data-parallel over N per domain: each core takes a contiguous 8192-sample
shard of every domain.

Phase 1: local per-class segment sums AND counts in one accumulating
matmul per 128-sample tile (the fp16 feature cache carries a ones column,
so the [C, D+1] partial needs no separate count matmuls).  Partials are
all-reduced (fp16) across the 8 cores.  While feature DMA streams, the
center-independent part of the distance -- per-sample ||f'||^2 -- is
computed in bulk (ACT Square + DVE 3D-reduce over 6-tile groups).

Phase 2 uses the expansion ||f - c_l||^2 = ||f'||^2 - 2*(f' . g') where
g' = onehot-gather of [centers | alpha], alpha = (1 - ||c||^2)/2 and
f' = [f | 1]: per tile ONE gather matmul (PSUM) plus ONE DVE
scalar_tensor_tensor dot with accumulator -- halving the PE rows of the
naive (gather - F)^2 scheme and keeping every engine's stream long and
independent.  Center-distance terms are finished on host from the tiny
all-reduced sums.
"""

import numpy as np

# Problem shape (hardcoded per contract).
N_DOM = 3
N = 65536
D = 256
C = 64
ALPHA, BETA, GAMA = 1.0, 1.0, 1.0
N_CORES = 8
NSH = N // N_CORES          # samples per core per domain
P = 128                     # partitions / tile height


def build(nsh=NSH, n_chunks=4):
    """Build + compile the SPMD Bass module. nsh = per-core samples/domain."""
    import concourse.bass as bass
    import concourse.bacc as bacc
    import concourse.mybir as mybir
    import concourse.tile as tile

    dt = mybir.dt
    Alu = mybir.AluOpType
    Act = mybir.ActivationFunctionType

    tiles = nsh // P                    # 128-sample tiles per domain (64)
    assert nsh % P == 0 and tiles % n_chunks == 0
    tpc = tiles // n_chunks             # tiles per DMA chunk
    DW = D + 1                          # feature cols + ones column
    G = 6                               # ||f'||^2 bulk group size

    nc = bacc.Bacc(
        "TRN2",
        target_bir_lowering=False,
        debug=False,
        num_devices=N_CORES,
    )

    feat = nc.dram_tensor("feat", [N_DOM, nsh, D], dt.float32, kind="ExternalInput")
    labels = nc.dram_tensor("labels", [N_DOM, nsh], dt.int32, kind="ExternalInput")
    out_sums = nc.dram_tensor(
        "out_sums", [N_DOM, C, DW], dt.float16, kind="ExternalOutput"
    )
    out_comp = nc.dram_tensor("out_comp", [N_DOM, 1], dt.float32, kind="ExternalOutput")

    rg = [list(range(N_CORES))]

    with tile.TileContext(nc) as tc:
        with (
            tc.tile_pool(name="persist", bufs=1) as pp,
            tc.tile_pool(name="lab", bufs=1) as labp,
            tc.tile_pool(name="ohgrp", bufs=3) as ohp,
            tc.tile_pool(name="cent", bufs=2) as centp,
            tc.tile_pool(name="sq", bufs=2) as sqp,
            tc.tile_pool(name="tr", bufs=2) as trp,
            tc.tile_pool(name="pseg", bufs=1, space="PSUM") as psegp,
            tc.tile_pool(name="plc", bufs=1, space="PSUM") as plcp,
            tc.tile_pool(name="pdy", bufs=6, space="PSUM") as pdyp,
            tc.tile_pool(name="dram", bufs=1, space="DRAM") as dramp,
        ):
            # ---- constants -------------------------------------------------
            iota64_i = pp.tile([P, C], dt.int16, tag="iota64_i")
            nc.gpsimd.iota(iota64_i[:], pattern=[[1, C]], base=0, channel_multiplier=0)
            iota64 = pp.tile([P, C], dt.float16, tag="iota64")
            nc.vector.tensor_copy(iota64[:], iota64_i[:])
            iota128 = pp.tile([P, P], dt.int16, tag="iota128")
            nc.gpsimd.iota(iota128[:], pattern=[[1, P]], base=0, channel_multiplier=0)
            pidx_i = pp.tile([P, 1], dt.int32, tag="pidx_i")
            nc.gpsimd.iota(pidx_i[:], pattern=[[0, 1]], base=0, channel_multiplier=1)
            pidx = pp.tile([P, 1], dt.float32, tag="pidx")
            nc.vector.tensor_copy(pidx[:], pidx_i[:])
            cidx_i = pp.tile([C, 1], dt.int32, tag="cidx_i")
            nc.gpsimd.iota(cidx_i[:], pattern=[[0, 1]], base=0, channel_multiplier=1)
            cidx = pp.tile([C, 1], dt.float32, tag="cidx")
            nc.vector.tensor_copy(cidx[:], cidx_i[:])
            posI = pp.tile([P, P], dt.float16, tag="posI")
            nc.vector.tensor_scalar(
                posI[:], iota128[:], pidx[:], None, Alu.is_equal
            )
            ones_col = pp.tile([P, 1], dt.float32, tag="ones_col")
            nc.vector.memset(ones_col[:], 1.0)

            # ---- persistent state -----------------------------------------
            # fp16 feature cache with a ones column per tile: [128, ND*T*257]
            f16 = pp.tile([P, N_DOM * tiles * DW], dt.float16, tag="f16")
            f16v = f16[:].rearrange("p (t m) -> p t m", m=DW)
            nc.vector.memset(f16v[:, :, D:DW], 1.0)
            # per-sample ||f'||^2 and f'.g' accumulators, one col per tile
            qf = pp.tile([P, N_DOM * tiles], dt.float32, tag="qf")
            qdot = pp.tile([P, N_DOM * tiles], dt.float32, tag="qdot")
            s_loc = [
                pp.tile([C, DW], dt.float16, tag=f"sloc{d}", name=f"sloc{d}")
                for d in range(N_DOM)
            ]
            s_glob = [
                pp.tile([C, DW], dt.float16, tag=f"sglob{d}", name=f"sglob{d}")
                for d in range(N_DOM)
            ]
            lab16 = [
                pp.tile([P, tiles], dt.float16, tag=f"lab16_{d}", name=f"lab16_{d}")
                for d in range(N_DOM)
            ]

            # ---- labels: DMA + PE transpose + bulk one-hot ----------------
            for d in range(N_DOM):
                lab_rows = labp.tile([tiles, P], dt.float16, tag="lab_rows", bufs=2)
                nc.gpsimd.dma_start(
                    lab_rows[:], labels[d].rearrange("(t p) -> t p", p=P)
                )
                plc = plcp.tile([P, tiles], dt.float16, tag="plc")
                nc.tensor.transpose(plc[:], lab_rows[:], posI[:tiles, :tiles])
                nc.vector.tensor_copy(lab16[d][:], plc[:])

            oh_all = []
            for d in range(N_DOM):
                # one-hot [128, tiles*C]: oh[p, t, c] = (labels[t*128+p] == c)
                oh = ohp.tile([P, tiles * C], dt.float16, tag="ohgrp", name=f"oh{d}")
                nc.vector.tensor_tensor(
                    oh[:].rearrange("p (t c) -> p t c", c=C),
                    lab16[d][:].rearrange("p (t o) -> p t o", o=1).broadcast_to(
                        (P, tiles, C)
                    ),
                    iota64[:].rearrange("p (o c) -> p o c", o=1).broadcast_to(
                        (P, tiles, C)
                    ),
                    Alu.is_equal,
                )
                oh_all.append(oh)

            # ---- helpers ---------------------------------------------------
            def feat_dma(d):
                dbase = d * tiles * DW
                for k in range(n_chunks):
                    src = feat[d, k * tpc * P:(k + 1) * tpc * P, :].rearrange(
                        "(t p) m -> p t m", p=P
                    )
                    dst = f16[
                        :, dbase + k * tpc * DW: dbase + (k + 1) * tpc * DW
                    ].rearrange("p (t m) -> p t m", m=DW)[:, :, 0:D]
                    nc.gpsimd.dma_start(dst, src)

            def bc_labels(d):
                # labels replicated across C partitions via a broadcast-read
                # cast DMA (no gpsimd compute op -> no microcode lib switch)
                lab_bc = labp.tile([C, nsh], dt.float16, tag="lab_bc")
                nc.gpsimd.dma_start(
                    lab_bc[:], labels[d:d + 1, :].broadcast_to((C, nsh))
                )
                return lab_bc

            ohT = [None] * N_DOM

            def build_ohT(d, lab_bc):
                # transposed one-hot [C, nsh] (reuses oh slot rotation)
                t_ohT = ohp.tile([C, nsh], dt.float16, tag="ohgrp", name=f"ohT{d}")
                nc.vector.tensor_scalar(
                    t_ohT[:], lab_bc[:], cidx[:], None, Alu.is_equal
                )
                ohT[d] = t_ohT

            cc_outs = [None] * N_DOM

            def p1(d):
                # segment sums + counts: one accumulating matmul per tile
                dbase = d * tiles * DW
                pseg = psegp.tile([C, DW], dt.float32, tag="pseg")
                for t in range(tiles):
                    nc.tensor.matmul(
                        pseg[:],
                        oh_all[d][:, t * C:(t + 1) * C],
                        f16[:, dbase + t * DW: dbase + (t + 1) * DW],
                        start=(t == 0),
                        stop=(t == tiles - 1),
                    )
                nc.scalar.copy(s_loc[d][:], pseg[:])
                cc_in = dramp.tile([C, DW], dt.float16, tag=f"ccin{d}", name=f"ccin{d}")
                cc_out = dramp.tile(
                    [C, DW], dt.float16, tag=f"ccout{d}", name=f"ccout{d}"
                )
                cc_outs[d] = cc_out
                nc.sync.dma_start(cc_in[:], s_loc[d][:])
                nc.gpsimd.collective_compute(
                    "AllReduce",
                    Alu.add,
                    replica_groups=rg,
                    ins=[cc_in.opt()],
                    outs=[cc_out.opt()],
                )

            def qf_bulk(d):
                # per-sample ||f'||^2 in 6-tile groups (center-independent,
                # runs in the DMA/AR-wait window on otherwise idle engines)
                dbase = d * tiles * DW
                t0 = 0
                while t0 < tiles:
                    g = min(G, tiles - t0)
                    sqf = sqp.tile([P, G * DW], dt.float16, tag="sq")
                    nc.scalar.activation(
                        sqf[:, 0:g * DW],
                        f16[:, dbase + t0 * DW: dbase + (t0 + g) * DW],
                        Act.Square,
                    )
                    nc.vector.tensor_reduce(
                        qf[:, d * tiles + t0: d * tiles + t0 + g],
                        sqf[:, 0:g * DW].rearrange("p (g m) -> p g m", m=DW),
                        axis=mybir.AxisListType.X,
                        op=Alu.add,
                    )
                    t0 += g

            def p2(d):
                # center-gated part: gather [centers | alpha] and dot with f'
                dbase = d * tiles * DW
                nc.scalar.dma_start(s_glob[d][:], cc_outs[d][:])
                cnt_cl = centp.tile([C, 1], dt.float32, tag="cnt_cl")
                nc.vector.tensor_scalar_max(cnt_cl[:], s_glob[d][:, D:DW], 1.0)
                inv = centp.tile([C, 1], dt.float32, tag="inv")
                nc.vector.reciprocal(inv[:], cnt_cl[:])
                centext = centp.tile([C, DW], dt.float16, tag="centext")
                nc.vector.tensor_scalar(
                    centext[:, 0:D], s_glob[d][:, 0:D], inv[:], None, Alu.mult
                )
                # alpha = (1 - ||c||^2) / 2
                csq = centp.tile([C, D], dt.float16, tag="csq")
                nc.scalar.activation(csq[:], centext[:, 0:D], Act.Square)
                cn = centp.tile([C, 1], dt.float32, tag="cn")
                nc.vector.tensor_reduce(
                    cn[:], csq[:], axis=mybir.AxisListType.X, op=Alu.add
                )
                nc.vector.tensor_scalar(
                    centext[:, D:DW], cn[:], -0.5, 0.5, Alu.mult, Alu.add
                )

                for t in range(tiles):
                    pdy = pdyp.tile([P, DW], dt.float32, tag="pdy")
                    nc.tensor.matmul(
                        pdy[:],
                        ohT[d][:, t * P:(t + 1) * P],
                        centext[:],
                        start=True,
                        stop=True,
                    )
                    tr = trp.tile([P, DW], dt.float16, tag="tr")
                    nc.vector.scalar_tensor_tensor(
                        tr[:],
                        pdy[:],
                        1.0,
                        f16[:, dbase + t * DW: dbase + (t + 1) * DW],
                        Alu.mult,
                        Alu.mult,
                        accum_out=qdot[:, d * tiles + t: d * tiles + t + 1],
                    )

            # ---- emission schedule ----------------------------------------
            feat_dma(0)
            bc0 = bc_labels(0)
            feat_dma(1)
            p1(0)
            qf_bulk(0)
            build_ohT(0, bc0)
            bc1 = bc_labels(1)
            feat_dma(2)
            p1(1)
            qf_bulk(1)
            build_ohT(1, bc1)
            bc2 = bc_labels(2)
            p1(2)
            qf_bulk(2)
            build_ohT(2, bc2)
            p2(0)
            p2(1)
            p2(2)

            # ---- finale: dist = sqrt(qf - 2*qdot); per-domain sums --------
            dist2 = pp.tile([P, N_DOM * tiles], dt.float32, tag="dist2")
            nc.vector.scalar_tensor_tensor(
                dist2[:], qdot[:], -2.0, qf[:], Alu.mult, Alu.add
            )
            dist = pp.tile([P, N_DOM * tiles], dt.float32, tag="dist")
            nc.scalar.activation(dist[:], dist2[:], Act.Sqrt)
            dsum = pp.tile([P, N_DOM], dt.float32, tag="dsum")
            nc.vector.tensor_reduce(
                dsum[:],
                dist[:].rearrange("p (d t) -> p d t", t=tiles),
                axis=mybir.AxisListType.X,
                op=Alu.add,
            )
            pc_t = plcp.tile([N_DOM, 1], dt.float32, tag="plc")
            nc.tensor.matmul(pc_t[:], dsum[:], ones_col[:], start=True, stop=True)
            comp_sb = pp.tile([N_DOM, 1], dt.float32, tag="comp_sb")
            nc.vector.tensor_copy(comp_sb[:], pc_t[:])
            nc.sync.dma_start(out_comp[:, :], comp_sb[:])
            for d in range(N_DOM):
                nc.sync.dma_start(out_sums[d], cc_outs[d][:])

    nc.compile()
    return nc


_CACHED = {}


def _get_nc(nsh=NSH, n_chunks=4):
    key = (nsh, n_chunks)
    if key not in _CACHED:
        _CACHED[key] = build(nsh, n_chunks)
    return _CACHED[key]


def finish_host(out_maps, n_total):
    """Combine per-core outputs into the scalar loss (numpy, float64)."""
    comp_sum = np.zeros(N_DOM, dtype=np.float64)
    for m in out_maps:
        comp_sum += m["out_comp"].reshape(-1).astype(np.float64)
    comp = comp_sum / n_total

    S = out_maps[0]["out_sums"].astype(np.float64)   # [N_DOM, C, D+1]
    sums, counts = S[:, :, :D], S[:, :, D]
    centers = sums / np.maximum(counts, 1.0)[:, :, None]

    sep = np.zeros(N_DOM, dtype=np.float64)
    for d in range(N_DOM):
        cd = centers[d]
        sq = ((cd[:, None, :] - cd[None, :, :]) ** 2).sum(-1)
        dist = np.sqrt(np.maximum(sq, 0.0))
        np.fill_diagonal(dist, 0.0)
        sep[d] = dist.sum() / (C * (C - 1))

    intra = (BETA * comp.sum() - ALPHA * sep.sum()) / N_DOM
    inter = 0.0
    n_pairs = 0
    for i in range(N_DOM):
        for j in range(i + 1, N_DOM):
            inter += np.sqrt(((centers[i] - centers[j]) ** 2).sum()) / C
            n_pairs += 1
    inter /= n_pairs
    return np.float32(GAMA * intra + inter)


def shard_inputs(features, labels, nsh):
    features = np.ascontiguousarray(np.asarray(features), dtype=np.float32)
    labels = np.ascontiguousarray(np.asarray(labels), dtype=np.int32)
    in_maps = []
    for c in range(N_CORES):
        in_maps.append({
            "feat": np.ascontiguousarray(features[:, c * nsh:(c + 1) * nsh, :]),
            "labels": np.ascontiguousarray(labels[:, c * nsh:(c + 1) * nsh]),
        })
    return in_maps


def kernel(features, labels):
    from concourse.bass_utils import run_bass_kernel_spmd

    nc = _get_nc()
    in_maps = shard_inputs(features, labels, NSH)
    res = run_bass_kernel_spmd(nc, in_maps, core_ids=list(range(N_CORES)))
    return finish_host(res.results, N)


# revision 16
# speedup vs baseline: 1.2031x; 1.2031x over previous
"""ClassAlignmentLoss kernel for 8 TRN2 NeuronCores (Bass/Tile).

Data-parallel over N per domain: each core takes a contiguous 8192-sample
shard of every domain.

Phase 1: local per-class segment sums AND counts in one accumulating
matmul per 128-sample tile (the fp16 feature cache carries a ones column,
so the [C, D+1] partial needs no separate count matmuls).  Partials are
all-reduced (fp16) across the 8 cores.  While feature DMA streams, the
center-independent part of the distance -- per-sample ||f'||^2 -- is
computed in bulk (ACT Square + DVE 3D-reduce over 6-tile groups).

Phase 2 uses the expansion ||f - c_l||^2 = ||f'||^2 - 2*(f' . g') where
g' = onehot-gather of [centers | alpha], alpha = (1 - ||c||^2)/2 and
f' = [f | 1]: per tile ONE gather matmul (PSUM) plus ONE DVE
scalar_tensor_tensor dot with accumulator -- halving the PE rows of the
naive (gather - F)^2 scheme and keeping every engine's stream long and
independent.  Center-distance terms are finished on host from the tiny
all-reduced sums.
"""

import numpy as np

# Problem shape (hardcoded per contract).
N_DOM = 3
N = 65536
D = 256
C = 64
ALPHA, BETA, GAMA = 1.0, 1.0, 1.0
N_CORES = 8
NSH = N // N_CORES          # samples per core per domain
P = 128                     # partitions / tile height


def build(nsh=NSH, n_chunks=4):
    """Build + compile the SPMD Bass module. nsh = per-core samples/domain."""
    import concourse.bass as bass
    import concourse.bacc as bacc
    import concourse.mybir as mybir
    import concourse.tile as tile

    dt = mybir.dt
    Alu = mybir.AluOpType
    Act = mybir.ActivationFunctionType

    tiles = nsh // P                    # 128-sample tiles per domain (64)
    assert nsh % P == 0 and tiles % n_chunks == 0
    tpc = tiles // n_chunks             # tiles per DMA chunk
    DW = D + 1                          # feature cols + ones column
    G = 6                               # ||f'||^2 bulk group size

    nc = bacc.Bacc(
        "TRN2",
        target_bir_lowering=False,
        debug=False,
        num_devices=N_CORES,
    )

    feat = nc.dram_tensor("feat", [N_DOM, nsh, D], dt.float32, kind="ExternalInput")
    labels = nc.dram_tensor("labels", [N_DOM, nsh], dt.int32, kind="ExternalInput")
    out_sums = nc.dram_tensor(
        "out_sums", [N_DOM, C, DW], dt.float16, kind="ExternalOutput"
    )
    out_comp = nc.dram_tensor("out_comp", [N_DOM, 1], dt.float32, kind="ExternalOutput")

    rg = [list(range(N_CORES))]

    with tile.TileContext(nc) as tc:
        with (
            tc.tile_pool(name="persist", bufs=1) as pp,
            tc.tile_pool(name="lab", bufs=1) as labp,
            tc.tile_pool(name="ohgrp", bufs=3) as ohp,
            tc.tile_pool(name="cent", bufs=2) as centp,
            tc.tile_pool(name="sq", bufs=2) as sqp,
            tc.tile_pool(name="tr", bufs=2) as trp,
            tc.tile_pool(name="pseg", bufs=1, space="PSUM") as psegp,
            tc.tile_pool(name="plc", bufs=1, space="PSUM") as plcp,
            tc.tile_pool(name="pdy", bufs=6, space="PSUM") as pdyp,
            tc.tile_pool(name="dram", bufs=1, space="DRAM") as dramp,
        ):
            # ---- constants -------------------------------------------------
            iota64_i = pp.tile([P, C], dt.int16, tag="iota64_i")
            nc.gpsimd.iota(iota64_i[:], pattern=[[1, C]], base=0, channel_multiplier=0)
            iota64 = pp.tile([P, C], dt.float16, tag="iota64")
            nc.vector.tensor_copy(iota64[:], iota64_i[:])
            iota128 = pp.tile([P, P], dt.int16, tag="iota128")
            nc.gpsimd.iota(iota128[:], pattern=[[1, P]], base=0, channel_multiplier=0)
            pidx_i = pp.tile([P, 1], dt.int32, tag="pidx_i")
            nc.gpsimd.iota(pidx_i[:], pattern=[[0, 1]], base=0, channel_multiplier=1)
            pidx = pp.tile([P, 1], dt.float32, tag="pidx")
            nc.vector.tensor_copy(pidx[:], pidx_i[:])
            cidx_i = pp.tile([C, 1], dt.int32, tag="cidx_i")
            nc.gpsimd.iota(cidx_i[:], pattern=[[0, 1]], base=0, channel_multiplier=1)
            cidx = pp.tile([C, 1], dt.float32, tag="cidx")
            nc.vector.tensor_copy(cidx[:], cidx_i[:])
            posI = pp.tile([P, P], dt.float16, tag="posI")
            nc.vector.tensor_scalar(
                posI[:], iota128[:], pidx[:], None, Alu.is_equal
            )
            ones_col = pp.tile([P, 1], dt.float32, tag="ones_col")
            nc.vector.memset(ones_col[:], 1.0)

            # ---- persistent state -----------------------------------------
            # fp16 feature cache with a ones column per tile: [128, ND*T*257]
            f16 = pp.tile([P, N_DOM * tiles * DW], dt.float16, tag="f16")
            f16v = f16[:].rearrange("p (t m) -> p t m", m=DW)
            nc.vector.memset(f16v[:, :, D:DW], 1.0)
            # per-sample ||f'||^2 and f'.g' accumulators, one col per tile
            qf = pp.tile([P, N_DOM * tiles], dt.float32, tag="qf")
            qdot = pp.tile([P, N_DOM * tiles], dt.float32, tag="qdot")
            s_loc = [
                pp.tile([C, DW], dt.float16, tag=f"sloc{d}", name=f"sloc{d}")
                for d in range(N_DOM)
            ]
            s_glob = [
                pp.tile([C, DW], dt.float16, tag=f"sglob{d}", name=f"sglob{d}")
                for d in range(N_DOM)
            ]
            lab16 = [
                pp.tile([P, tiles], dt.float16, tag=f"lab16_{d}", name=f"lab16_{d}")
                for d in range(N_DOM)
            ]

            # ---- labels: DMA + PE transpose + bulk one-hot ----------------
            for d in range(N_DOM):
                lab_rows = labp.tile([tiles, P], dt.float16, tag="lab_rows", bufs=2)
                nc.gpsimd.dma_start(
                    lab_rows[:], labels[d].rearrange("(t p) -> t p", p=P)
                )
                plc = plcp.tile([P, tiles], dt.float16, tag="plc")
                nc.tensor.transpose(plc[:], lab_rows[:], posI[:tiles, :tiles])
                nc.vector.tensor_copy(lab16[d][:], plc[:])

            oh_all = []
            for d in range(N_DOM):
                # one-hot [128, tiles*C]: oh[p, t, c] = (labels[t*128+p] == c)
                oh = ohp.tile([P, tiles * C], dt.float16, tag="ohgrp", name=f"oh{d}")
                nc.vector.tensor_tensor(
                    oh[:].rearrange("p (t c) -> p t c", c=C),
                    lab16[d][:].rearrange("p (t o) -> p t o", o=1).broadcast_to(
                        (P, tiles, C)
                    ),
                    iota64[:].rearrange("p (o c) -> p o c", o=1).broadcast_to(
                        (P, tiles, C)
                    ),
                    Alu.is_equal,
                )
                oh_all.append(oh)

            # ---- helpers ---------------------------------------------------
            def feat_dma(d):
                dbase = d * tiles * DW
                for k in range(n_chunks):
                    src = feat[d, k * tpc * P:(k + 1) * tpc * P, :].rearrange(
                        "(t p) m -> p t m", p=P
                    )
                    dst = f16[
                        :, dbase + k * tpc * DW: dbase + (k + 1) * tpc * DW
                    ].rearrange("p (t m) -> p t m", m=DW)[:, :, 0:D]
                    nc.gpsimd.dma_start(dst, src)

            def bc_labels(d):
                # labels replicated across C partitions via a broadcast-read
                # cast DMA (no gpsimd compute op -> no microcode lib switch)
                lab_bc = labp.tile([C, nsh], dt.float16, tag="lab_bc")
                nc.gpsimd.dma_start(
                    lab_bc[:], labels[d:d + 1, :].broadcast_to((C, nsh))
                )
                return lab_bc

            ohT = [None] * N_DOM

            def build_ohT(d, lab_bc):
                # transposed one-hot [C, nsh] (reuses oh slot rotation)
                t_ohT = ohp.tile([C, nsh], dt.float16, tag="ohgrp", name=f"ohT{d}")
                nc.vector.tensor_scalar(
                    t_ohT[:], lab_bc[:], cidx[:], None, Alu.is_equal
                )
                ohT[d] = t_ohT

            cc_outs = [None] * N_DOM

            def p1(d):
                # segment sums + counts: one accumulating matmul per tile
                dbase = d * tiles * DW
                pseg = psegp.tile([C, DW], dt.float32, tag="pseg")
                for t in range(tiles):
                    nc.tensor.matmul(
                        pseg[:],
                        oh_all[d][:, t * C:(t + 1) * C],
                        f16[:, dbase + t * DW: dbase + (t + 1) * DW],
                        start=(t == 0),
                        stop=(t == tiles - 1),
                    )
                nc.scalar.copy(s_loc[d][:], pseg[:])
                cc_in = dramp.tile([C, DW], dt.float16, tag=f"ccin{d}", name=f"ccin{d}")
                cc_out = dramp.tile(
                    [C, DW], dt.float16, tag=f"ccout{d}", name=f"ccout{d}"
                )
                cc_outs[d] = cc_out
                nc.sync.dma_start(cc_in[:], s_loc[d][:])
                nc.gpsimd.collective_compute(
                    "AllReduce",
                    Alu.add,
                    replica_groups=rg,
                    ins=[cc_in.opt()],
                    outs=[cc_out.opt()],
                )

            def qf_bulk(d):
                # per-sample ||f'||^2 in 6-tile groups (center-independent,
                # runs in the DMA/AR-wait window on otherwise idle engines)
                dbase = d * tiles * DW
                t0 = 0
                while t0 < tiles:
                    g = min(G, tiles - t0)
                    sqf = sqp.tile([P, G * DW], dt.float16, tag="sq")
                    nc.scalar.activation(
                        sqf[:, 0:g * DW],
                        f16[:, dbase + t0 * DW: dbase + (t0 + g) * DW],
                        Act.Square,
                    )
                    nc.vector.tensor_reduce(
                        qf[:, d * tiles + t0: d * tiles + t0 + g],
                        sqf[:, 0:g * DW].rearrange("p (g m) -> p g m", m=DW),
                        axis=mybir.AxisListType.X,
                        op=Alu.add,
                    )
                    t0 += g

            def p2(d):
                # center-gated part: gather [centers | alpha] and dot with f'
                dbase = d * tiles * DW
                nc.scalar.dma_start(s_glob[d][:], cc_outs[d][:])
                cnt_cl = centp.tile([C, 1], dt.float32, tag="cnt_cl")
                nc.vector.tensor_scalar_max(cnt_cl[:], s_glob[d][:, D:DW], 1.0)
                inv = centp.tile([C, 1], dt.float32, tag="inv")
                nc.vector.reciprocal(inv[:], cnt_cl[:])
                centext = centp.tile([C, DW], dt.float16, tag="centext")
                nc.vector.tensor_scalar(
                    centext[:, 0:D], s_glob[d][:, 0:D], inv[:], None, Alu.mult
                )
                # alpha = (1 - ||c||^2) / 2
                csq = centp.tile([C, D], dt.float16, tag="csq")
                nc.scalar.activation(csq[:], centext[:, 0:D], Act.Square)
                cn = centp.tile([C, 1], dt.float32, tag="cn")
                nc.vector.tensor_reduce(
                    cn[:], csq[:], axis=mybir.AxisListType.X, op=Alu.add
                )
                nc.vector.tensor_scalar(
                    centext[:, D:DW], cn[:], -0.5, 0.5, Alu.mult, Alu.add
                )

                for t in range(tiles):
                    pdy = pdyp.tile([P, DW], dt.float32, tag="pdy")
                    nc.tensor.matmul(
                        pdy[:],
                        ohT[d][:, t * P:(t + 1) * P],
                        centext[:],
                        start=True,
                        stop=True,
                    )
                    tr = trp.tile([P, DW], dt.float16, tag="tr")
                    nc.vector.scalar_tensor_tensor(
                        tr[:],
                        pdy[:],
                        1.0,
                        f16[:, dbase + t * DW: dbase + (t + 1) * DW],
                        Alu.mult,
                        Alu.mult,
                        accum_out=qdot[:, d * tiles + t: d * tiles + t + 1],
                    )

            # ---- emission schedule ----------------------------------------
            # NOTE: a collective trigger on gpsimd blocks its queue while a
            # previous collective is still in flight -- all DMA issues and
            # broadcasts must be emitted before the second AR trigger.
            feat_dma(0)
            bc0 = bc_labels(0)
            feat_dma(1)
            p1(0)
            qf_bulk(0)
            build_ohT(0, bc0)
            bc1 = bc_labels(1)
            feat_dma(2)
            bc2 = bc_labels(2)
            p1(1)
            qf_bulk(1)
            build_ohT(1, bc1)
            p1(2)
            qf_bulk(2)
            build_ohT(2, bc2)
            p2(0)
            p2(1)
            p2(2)

            # ---- finale: dist = sqrt(qf - 2*qdot); per-domain sums --------
            dist2 = pp.tile([P, N_DOM * tiles], dt.float32, tag="dist2")
            nc.vector.scalar_tensor_tensor(
                dist2[:], qdot[:], -2.0, qf[:], Alu.mult, Alu.add
            )
            dist = pp.tile([P, N_DOM * tiles], dt.float32, tag="dist")
            nc.scalar.activation(dist[:], dist2[:], Act.Sqrt)
            dsum = pp.tile([P, N_DOM], dt.float32, tag="dsum")
            nc.vector.tensor_reduce(
                dsum[:],
                dist[:].rearrange("p (d t) -> p d t", t=tiles),
                axis=mybir.AxisListType.X,
                op=Alu.add,
            )
            pc_t = plcp.tile([N_DOM, 1], dt.float32, tag="plc")
            nc.tensor.matmul(pc_t[:], dsum[:], ones_col[:], start=True, stop=True)
            comp_sb = pp.tile([N_DOM, 1], dt.float32, tag="comp_sb")
            nc.vector.tensor_copy(comp_sb[:], pc_t[:])
            nc.sync.dma_start(out_comp[:, :], comp_sb[:])
            for d in range(N_DOM):
                nc.sync.dma_start(out_sums[d], cc_outs[d][:])

    nc.compile()
    return nc


_CACHED = {}


def _get_nc(nsh=NSH, n_chunks=4):
    key = (nsh, n_chunks)
    if key not in _CACHED:
        _CACHED[key] = build(nsh, n_chunks)
    return _CACHED[key]


def finish_host(out_maps, n_total):
    """Combine per-core outputs into the scalar loss (numpy, float64)."""
    comp_sum = np.zeros(N_DOM, dtype=np.float64)
    for m in out_maps:
        comp_sum += m["out_comp"].reshape(-1).astype(np.float64)
    comp = comp_sum / n_total

    S = out_maps[0]["out_sums"].astype(np.float64)   # [N_DOM, C, D+1]
    sums, counts = S[:, :, :D], S[:, :, D]
    centers = sums / np.maximum(counts, 1.0)[:, :, None]

    sep = np.zeros(N_DOM, dtype=np.float64)
    for d in range(N_DOM):
        cd = centers[d]
        sq = ((cd[:, None, :] - cd[None, :, :]) ** 2).sum(-1)
        dist = np.sqrt(np.maximum(sq, 0.0))
        np.fill_diagonal(dist, 0.0)
        sep[d] = dist.sum() / (C * (C - 1))

    intra = (BETA * comp.sum() - ALPHA * sep.sum()) / N_DOM
    inter = 0.0
    n_pairs = 0
    for i in range(N_DOM):
        for j in range(i + 1, N_DOM):
            inter += np.sqrt(((centers[i] - centers[j]) ** 2).sum()) / C
            n_pairs += 1
    inter /= n_pairs
    return np.float32(GAMA * intra + inter)


def shard_inputs(features, labels, nsh):
    features = np.ascontiguousarray(np.asarray(features), dtype=np.float32)
    labels = np.ascontiguousarray(np.asarray(labels), dtype=np.int32)
    in_maps = []
    for c in range(N_CORES):
        in_maps.append({
            "feat": np.ascontiguousarray(features[:, c * nsh:(c + 1) * nsh, :]),
            "labels": np.ascontiguousarray(labels[:, c * nsh:(c + 1) * nsh]),
        })
    return in_maps


def kernel(features, labels):
    from concourse.bass_utils import run_bass_kernel_spmd

    nc = _get_nc()
    in_maps = shard_inputs(features, labels, NSH)
    res = run_bass_kernel_spmd(nc, in_maps, core_ids=list(range(N_CORES)))
    return finish_host(res.results, N)
